# revision 12
# baseline (speedup 1.0000x reference)
"""MultiHeadAttention (B=4, S=2048, D=2048, H=16) on 8 TRN2 NeuronCores.

Sharding: core c handles batch b = c//2 and head-half = c%2 (8 heads).
Each core computes Q/K/V projections for its 1024 rows, attention for its
8 heads, and a partial output projection; the host sums the two partials
per batch and un-permutes.

Layout trick: torch's `view(B, H, S, dk)` head split (no transpose) means
head h of batch b lives in rows [128h, 128h+128) of the projection output,
with each row holding 16 consecutive seq positions. Working in permuted
query/key coordinates pi = 128*t + u (s = 16*u + t), every attention
operand is an exact 128x128 tile of either the transposed projection
(R^T, for Q/K) or the natural projection (R, for V). Softmax is
permutation-invariant, and the host un-permutes the final output.

All matmuls run in float32r (fp32 with 10-bit mantissa, full PE speed);
host pre-rounds all external matmul operands. Weights are pre-tiled on
the host for contiguous loads; every transfer >256KB is split across DMA
queues. Projection input stages rotate through one double-buffered pool.
"""
import math
import os
from contextlib import ExitStack

import numpy as np

B, S, D, H = 4, 2048, 2048, 16
DK = D // H            # 128
HPC = H // 2           # heads per core = 8
RPC = HPC * DK         # rows per core = 1024
NC_ = 8                # cores
MC = D // 128          # contraction chunks = 16
SCALE = 1.0 / math.sqrt(DK)

_cache = {}
last_results = None


def _round_f32r(x):
    """Round fp32 to the 10-bit-mantissa grid the PE uses for float32r."""
    x = np.ascontiguousarray(x, dtype=np.float32)
    u = x.view(np.uint32)
    lsb = (u >> np.uint32(13)) & np.uint32(1)
    r = (u + np.uint32(0x0FFF) + lsb) & np.uint32(0xFFFFE000)
    return r.view(np.float32)


def _build():
    import concourse.bass as bass
    import concourse.mybir as mybir
    import concourse.tile as tile
    from concourse import bacc

    f32 = mybir.dt.float32
    f32r = mybir.dt.float32r
    AF = mybir.ActivationFunctionType

    nc = bacc.Bacc("TRN2", target_bir_lowering=False, debug=False,
                   num_devices=NC_)

    # ---- external I/O ----
    qts_d = nc.dram_tensor("qts", (MC, 128, RPC), f32r, kind="ExternalInput")
    kts_d = nc.dram_tensor("kts", (MC, 128, RPC), f32r, kind="ExternalInput")
    vts_d = nc.dram_tensor("vts", (MC, 128, RPC), f32r, kind="ExternalInput")
    wqt_d = nc.dram_tensor("wqt", (MC, 128, MC, 128), f32r, kind="ExternalInput")
    wkt_d = nc.dram_tensor("wkt", (MC, 128, MC, 128), f32r, kind="ExternalInput")
    wvt_d = nc.dram_tensor("wvt", (8, 128, MC, 256), f32r, kind="ExternalInput")
    wot_d = nc.dram_tensor("wot", (MC, 128, HPC, 128), f32r, kind="ExternalInput")
    bqs_d = nc.dram_tensor("bqs", (D,), f32, kind="ExternalInput")
    bk_d = nc.dram_tensor("bk", (D,), f32, kind="ExternalInput")
    bvr_d = nc.dram_tensor("bvr", (1, D), f32r, kind="ExternalInput")
    bo_d = nc.dram_tensor("bo", (D,), f32, kind="ExternalInput")
    ones1_d = nc.dram_tensor("ones1", (1, 128), f32r, kind="ExternalInput")
    onescol_d = nc.dram_tensor("onescol", (128, 1), f32r, kind="ExternalInput")
    out_d = nc.dram_tensor("out", (D, S), f32, kind="ExternalOutput")

    with tile.TileContext(nc) as tc, ExitStack() as top:
        rpool = top.enter_context(tc.tile_pool(name="consts", bufs=1))
        dpool = top.enter_context(tc.tile_pool(name="dram", bufs=1, space="DRAM"))

        bq_sb = rpool.tile([128, MC], f32)
        bk_sb = rpool.tile([128, MC], f32)
        bo_sb = rpool.tile([128, MC], f32)
        bv_sb = rpool.tile([1, D], f32r)
        ones1 = rpool.tile([1, 128], f32r)
        onescol = rpool.tile([128, 1], f32r)
        nc.sync.dma_start(bq_sb[:], bqs_d.ap().rearrange("(t p) -> p t", p=128))
        nc.sync.dma_start(bk_sb[:], bk_d.ap().rearrange("(t p) -> p t", p=128))
        nc.sync.dma_start(bo_sb[:], bo_d.ap().rearrange("(t p) -> p t", p=128))
        nc.sync.dma_start(bv_sb[:], bvr_d.ap())
        nc.sync.dma_start(ones1[:], ones1_d.ap())
        nc.sync.dma_start(onescol[:], onescol_d.ap())

        qhat_dram = dpool.tile([MC, 128, RPC], f32r)        # [t][dk][r]
        khat_dram = dpool.tile([HPC, 128, MC, 128], f32r)   # [h][dk][tk][u]
        vhat_dram = dpool.tile([RPC, D], f32r)              # natural R_v

        def load_stage(pool, src_d):
            st = pool.tile([128, MC, RPC], f32r, tag="st")
            for mc in range(MC):
                nc.sync.dma_start(st[:, mc, :], src_d.ap()[mc])
            return st

        def load_w16(pool, src_ap, tag):
            """Load a [128, MC, 128] weight tile in 4 mc-chunks."""
            st = pool.tile([128, MC, 128], f32r, tag=tag)
            for g in range(4):
                nc.sync.dma_start(st[:, 4 * g:4 * g + 4, :],
                                  src_ap[:, 4 * g:4 * g + 4, :])
            return st

        with ExitStack() as stages_es:
            st_pool = stages_es.enter_context(tc.tile_pool(name="stages", bufs=2))

            # ============= phase Q (first; weights load from t=0) ======
            with ExitStack() as ph:
                wq_pool = ph.enter_context(tc.tile_pool(name="wq", bufs=4))
                qps_pool = ph.enter_context(
                    tc.tile_pool(name="qps", bufs=4, space="PSUM"))
                qout_pool = ph.enter_context(tc.tile_pool(name="qout", bufs=4))
                with nc.named_scope("proj_q"):
                    qt_st = load_stage(st_pool, qts_d)
                    vt_st = load_stage(st_pool, vts_d)   # prefetch for V
                    for ct in range(MC):
                        wq_st = load_w16(wq_pool, wqt_d.ap()[ct], "wq")
                        for rb in range(2):
                            ps = qps_pool.tile([128, 512], f32, tag="qps")
                            for mc in range(MC):
                                nc.tensor.matmul(
                                    ps[:], wq_st[:, mc, :],
                                    qt_st[:, mc, 512 * rb:512 * rb + 512],
                                    start=(mc == 0), stop=(mc == MC - 1))
                            qo = qout_pool.tile([128, 512], f32r, tag="qo")
                            nc.scalar.activation(qo[:], ps[:], AF.Identity,
                                                 bias=bq_sb[:, ct:ct + 1],
                                                 scale=SCALE)
                            nc.gpsimd.dma_start(
                                qhat_dram[ct, :, 512 * rb:512 * rb + 512],
                                qo[:])

            # ============= phase V (+ prefetch K stage) ================
            with ExitStack() as ph:
                wv_pool = ph.enter_context(tc.tile_pool(name="wv", bufs=2))
                vps_pool = ph.enter_context(
                    tc.tile_pool(name="vps", bufs=4, space="PSUM"))
                vout_pool = ph.enter_context(tc.tile_pool(name="vout", bufs=4))
                with nc.named_scope("proj_v"):
                    kt_st = load_stage(st_pool, kts_d)   # prefetch (rotates)
                    for cb in range(8):           # c blocks of 256
                        wv_st = wv_pool.tile([128, MC, 256], f32r, tag="wv")
                        for g in range(4):
                            nc.sync.dma_start(
                                wv_st[:, 4 * g:4 * g + 4, :],
                                wvt_d.ap()[cb][:, 4 * g:4 * g + 4, :])
                        for rt in range(8):       # r tiles of 128
                            ps = vps_pool.tile([128, 256], f32, tag="vps")
                            for mc in range(MC):
                                nc.tensor.matmul(
                                    ps[:],
                                    vt_st[:, mc, 128 * rt:128 * rt + 128],
                                    wv_st[:, mc, :], start=(mc == 0),
                                    stop=False)
                            nc.tensor.matmul(
                                ps[:], ones1[:],
                                bv_sb[:, 256 * cb:256 * cb + 256],
                                start=False, stop=True)
                            vo = vout_pool.tile([128, 256], f32r, tag="vo")
                            nc.vector.tensor_copy(vo[:], ps[:])
                            nc.gpsimd.dma_start(
                                vhat_dram[128 * rt:128 * rt + 128,
                                          256 * cb:256 * cb + 256], vo[:])

            # ============= phase K -> khat_dram [h][dk][tk][u] =========
            with ExitStack() as ph:
                wk_pool = ph.enter_context(tc.tile_pool(name="wk", bufs=4))
                kps_pool = ph.enter_context(
                    tc.tile_pool(name="kps", bufs=4, space="PSUM"))
                kout_pool = ph.enter_context(tc.tile_pool(name="kout", bufs=4))
                with nc.named_scope("proj_k"):
                    for ct in range(MC):
                        wk_st = load_w16(wk_pool, wkt_d.ap()[ct], "wk")
                        for rb in range(2):
                            ps = kps_pool.tile([128, 512], f32, tag="kps")
                            for mc in range(MC):
                                nc.tensor.matmul(
                                    ps[:], wk_st[:, mc, :],
                                    kt_st[:, mc, 512 * rb:512 * rb + 512],
                                    start=(mc == 0), stop=(mc == MC - 1))
                            ko = kout_pool.tile([128, 4, 128], f32r, tag="ko")
                            nc.scalar.activation(ko[:], ps[:], AF.Identity,
                                                 bias=bk_sb[:, ct:ct + 1],
                                                 scale=1.0)
                            dst = khat_dram[4 * rb:4 * rb + 4, :, ct, :] \
                                .rearrange("h p u -> p h u")
                            nc.gpsimd.dma_start(dst, ko[:])

        # ============= attention + output projection ===============
        with ExitStack() as ph:
            q_pool = ph.enter_context(tc.tile_pool(name="qrhs", bufs=4))
            kh_pool = ph.enter_context(tc.tile_pool(name="kh", bufs=3))
            vh_pool = ph.enter_context(tc.tile_pool(name="vh", bufs=3))
            exp_pool = ph.enter_context(tc.tile_pool(name="expp", bufs=6))
            tree_pool = ph.enter_context(tc.tile_pool(name="tree", bufs=2))
            scps_pool = ph.enter_context(
                tc.tile_pool(name="scps", bufs=2, space="PSUM"))
            xps_pool = ph.enter_context(
                tc.tile_pool(name="xps", bufs=2, space="PSUM"))
            sps_pool = ph.enter_context(
                tc.tile_pool(name="sps", bufs=1, space="PSUM"))
            ops_pool = ph.enter_context(
                tc.tile_pool(name="ops", bufs=1, space="PSUM"))
            nrm_pool = ph.enter_context(tc.tile_pool(name="nrm", bufs=2))
            x_pool = ph.enter_context(tc.tile_pool(name="xsb", bufs=2))
            wo_pool = ph.enter_context(tc.tile_pool(name="wo", bufs=3))
            oout_pool = ph.enter_context(tc.tile_pool(name="oout", bufs=3))
            with nc.named_scope("attn"):
                NP_ = MC // 2     # key-tile pairs per head

                def attn_block(j, h, x_j, k_h, v_h):
                    q_rhs = q_pool.tile([128, 4, 128], f32r, tag="qr")
                    nc.sync.dma_start(
                        q_rhs[:],
                        qhat_dram[4 * j:4 * j + 4, :, 128 * h:128 * h + 128]
                        .rearrange("t p u -> p t u"))
                    x_ps = xps_pool.tile([128, 512], f32, tag="xps")
                    s_ps = sps_pool.tile([1, 512], f32, tag="sps")
                    acc = tree_pool.tile([128, 512], f32, tag="acc")
                    tsum = tree_pool.tile([128, 512], f32r, tag="tf")
                    exs = [None] * NP_
                    t1s = [None] * NP_

                    def pv_and_sum(tp):
                        ex = exs[tp]
                        for i in range(2):
                            nc.tensor.matmul(
                                x_ps[:], v_h[:, 2 * tp + i, :],
                                ex[:, i, :], start=(tp == 0 and i == 0),
                                stop=(tp == NP_ - 1 and i == 1))
                        t1 = tree_pool.tile([128, 512], f32, tag="t1")
                        nc.vector.tensor_add(
                            t1[:], ex[:, 0, :].bitcast(f32),
                            ex[:, 1, :].bitcast(f32))
                        t1s[tp] = t1
                        if tp == 1:
                            nc.vector.tensor_add(acc[:], t1s[0][:], t1[:])
                        elif 1 < tp < NP_ - 1:
                            nc.vector.tensor_add(acc[:], acc[:], t1[:])
                        elif tp == NP_ - 1:
                            nc.vector.tensor_add(tsum[:], acc[:], t1[:])

                    for tp in range(NP_):
                        sc = scps_pool.tile([128, 2, 512], f32, tag="sc")
                        for i in range(2):
                            tk = 2 * tp + i
                            nc.tensor.matmul(
                                sc[:, i, :], k_h[:, tk, :],
                                q_rhs[:, :, :], start=True, stop=True)
                        ex = exp_pool.tile([128, 2, 512], f32r, tag="ex")
                        nc.scalar.activation(ex[:], sc[:], AF.Exp, scale=1.0)
                        exs[tp] = ex
                        if tp >= 1:
                            pv_and_sum(tp - 1)
                    pv_and_sum(NP_ - 1)

                    nc.tensor.matmul(s_ps[:], onescol[:], tsum[:],
                                     start=True, stop=True)
                    rec = nrm_pool.tile([1, 512], f32, tag="rec")
                    nc.vector.reciprocal_approx_fast(rec[:], s_ps[:])
                    bcast = nrm_pool.tile([128, 512], f32, tag="bc")
                    nc.gpsimd.partition_broadcast(bcast[:], rec[:])
                    nc.vector.tensor_mul(x_j[:, h, :], x_ps[:], bcast[:])

                def emit_outproj(j, x_j):
                    for ot in range(MC):
                        wo_st = wo_pool.tile([128, HPC, 128], f32r, tag="wo")
                        for g in range(2):
                            nc.sync.dma_start(
                                wo_st[:, 4 * g:4 * g + 4, :],
                                wot_d.ap()[ot][:, 4 * g:4 * g + 4, :])
                        op = ops_pool.tile([128, 512], f32, tag="op")
                        for h in range(HPC):
                            nc.tensor.matmul(op[:], wo_st[:, h, :],
                                             x_j[:, h, :], start=(h == 0),
                                             stop=(h == HPC - 1))
                        oo = oout_pool.tile([128, 512], f32, tag="oo")
                        nc.scalar.activation(oo[:], op[:], AF.Identity,
                                             bias=bo_sb[:, ot:ot + 1],
                                             scale=1.0)
                        nc.gpsimd.dma_start(
                            out_d.ap()[128 * ot:128 * ot + 128,
                                       512 * j:512 * j + 512], oo[:])

                for jp in range(2):           # pairs of query pi-blocks
                    j0, j1 = 2 * jp, 2 * jp + 1
                    x_j0 = x_pool.tile([128, HPC, 512], f32r, tag="xj")
                    x_j1 = x_pool.tile([128, HPC, 512], f32r, tag="xj")
                    for h in range(HPC):
                        k_h = kh_pool.tile([128, MC, 128], f32r, tag="kh")
                        for g in range(4):
                            nc.sync.dma_start(
                                k_h[:, 4 * g:4 * g + 4, :],
                                khat_dram[h][:, 4 * g:4 * g + 4, :])
                        v_h = vh_pool.tile([128, MC, 128], f32r, tag="vh")
                        for g in range(4):
                            nc.sync.dma_start(
                                v_h[:, 4 * g:4 * g + 4, :],
                                vhat_dram[128 * h:128 * h + 128,
                                          512 * g:512 * g + 512])
                        attn_block(j0, h, x_j0, k_h, v_h)
                        attn_block(j1, h, x_j1, k_h, v_h)
                    emit_outproj(j0, x_j0)
                    emit_outproj(j1, x_j1)

    nc.compile()
    return nc


def _prep_shared(Wq, Wk, Wv, Wo, bq, bk, bv, bo):
    wqt = _round_f32r(np.ascontiguousarray(np.asarray(Wq, np.float32).T))
    wkt = _round_f32r(np.ascontiguousarray(np.asarray(Wk, np.float32).T))
    wvt = _round_f32r(np.ascontiguousarray(np.asarray(Wv, np.float32).T))
    wqt_t = np.ascontiguousarray(
        wqt.reshape(MC, 128, MC, 128).transpose(2, 1, 0, 3))
    wkt_t = np.ascontiguousarray(
        wkt.reshape(MC, 128, MC, 128).transpose(2, 1, 0, 3))
    wvt_t = np.ascontiguousarray(
        wvt.reshape(MC, 128, 8, 256).transpose(2, 1, 0, 3))
    woT = np.ascontiguousarray(np.asarray(Wo, np.float32).T)
    bqs = (np.asarray(bq, np.float32) * SCALE).copy()
    bk_np = np.asarray(bk, np.float32).copy()
    bvr = _round_f32r(np.asarray(bv, np.float32).reshape(1, D))
    bo_np = np.asarray(bo, np.float32).copy()
    return wqt_t, wkt_t, wvt_t, woT, bqs, bk_np, bvr, bo_np


def kernel(Q, K, V, Wq, bq, Wk, bk, Wv, bv, Wo, bo, num_heads):
    global last_results
    assert int(num_heads) == H

    from concourse.bass_utils import run_bass_kernel_spmd

    if "nc" not in _cache:
        _cache["nc"] = _build()
    nc = _cache["nc"]

    Q = np.asarray(Q, np.float32)
    K = np.asarray(K, np.float32)
    V = np.asarray(V, np.float32)
    wqt_t, wkt_t, wvt_t, woT, bqs, bk_np, bvr, bo_np = _prep_shared(
        Wq, Wk, Wv, Wo, bq, bk, bv, bo)
    ones1 = np.ones((1, 128), np.float32)
    onescol = np.ones((128, 1), np.float32)

    in_maps = []
    for c in range(NC_):
        b, half = divmod(c, 2)
        r0 = RPC * half
        wot_t = np.ascontiguousarray(
            _round_f32r(woT[r0:r0 + RPC, :])
            .reshape(HPC, 128, MC, 128).transpose(2, 1, 0, 3))
        in_maps.append({
            "qts": _round_f32r(Q[b].T[:, r0:r0 + RPC]).reshape(MC, 128, RPC),
            "kts": _round_f32r(K[b].T[:, r0:r0 + RPC]).reshape(MC, 128, RPC),
            "vts": _round_f32r(V[b].T[:, r0:r0 + RPC]).reshape(MC, 128, RPC),
            "wqt": wqt_t, "wkt": wkt_t, "wvt": wvt_t, "wot": wot_t,
            "bqs": bqs, "bk": bk_np, "bvr": bvr, "bo": bo_np,
            "ones1": ones1, "onescol": onescol,
        })

    res = run_bass_kernel_spmd(nc, in_maps, core_ids=list(range(NC_)))
    last_results = res

    out = np.empty((B, S, D), np.float32)
    for b in range(B):
        oT = res.results[2 * b]["out"] + res.results[2 * b + 1]["out"]
        # oT[o, pi], pi = 128*t + u ; s = 16*u + t
        out[b] = oT.reshape(D, 16, 128).transpose(2, 1, 0).reshape(S, D)
    return out


# revision 13
# speedup vs baseline: 1.2201x; 1.2201x over previous
"""MultiHeadAttention (B=4, S=2048, D=2048, H=16) on 8 TRN2 NeuronCores.

Sharding: core c handles batch b = c//2 and head-half = c%2 (8 heads).
Each core computes Q/K/V projections for its 1024 rows, attention for its
8 heads, and a partial output projection; the host sums the two partials
per batch and un-permutes.

Layout trick: torch's `view(B, H, S, dk)` head split (no transpose) means
head h of batch b lives in rows [128h, 128h+128) of the projection output,
with each row holding 16 consecutive seq positions. Working in permuted
query/key coordinates pi = 128*t + u (s = 16*u + t), every attention
operand is an exact 128x128 tile of either the transposed projection
(R^T, for Q/K) or the natural projection (R, for V). Softmax is
permutation-invariant, and the host un-permutes the final output.

All matmuls run in float32r (fp32 with 10-bit mantissa, full PE speed);
host pre-rounds all external matmul operands. Weights are pre-tiled on
the host for contiguous loads; every transfer >256KB is split across DMA
queues. Projection input stages rotate through one double-buffered pool.
"""
import math
import os
from contextlib import ExitStack

import numpy as np

B, S, D, H = 4, 2048, 2048, 16
DK = D // H            # 128
HPC = H // 2           # heads per core = 8
RPC = HPC * DK         # rows per core = 1024
NC_ = 8                # cores
MC = D // 128          # contraction chunks = 16
SCALE = 1.0 / math.sqrt(DK)

_cache = {}
last_results = None


def _round_f32r(x):
    """Round fp32 to the 10-bit-mantissa grid the PE uses for float32r."""
    x = np.ascontiguousarray(x, dtype=np.float32)
    u = x.view(np.uint32)
    lsb = (u >> np.uint32(13)) & np.uint32(1)
    r = (u + np.uint32(0x0FFF) + lsb) & np.uint32(0xFFFFE000)
    return r.view(np.float32)


def _build():
    import concourse.bass as bass
    import concourse.mybir as mybir
    import concourse.tile as tile
    from concourse import bacc

    f32 = mybir.dt.float32
    f32r = mybir.dt.float32r
    AF = mybir.ActivationFunctionType

    nc = bacc.Bacc("TRN2", target_bir_lowering=False, debug=False,
                   num_devices=NC_)

    # ---- external I/O ----
    qts_d = nc.dram_tensor("qts", (MC, 128, RPC), f32r, kind="ExternalInput")
    kts_d = nc.dram_tensor("kts", (MC, 128, RPC), f32r, kind="ExternalInput")
    vts_d = nc.dram_tensor("vts", (MC, 128, RPC), f32r, kind="ExternalInput")
    wqt_d = nc.dram_tensor("wqt", (MC, 128, MC, 128), f32r, kind="ExternalInput")
    wkt_d = nc.dram_tensor("wkt", (MC, 128, MC, 128), f32r, kind="ExternalInput")
    wvt_d = nc.dram_tensor("wvt", (8, 128, MC, 256), f32r, kind="ExternalInput")
    wot_d = nc.dram_tensor("wot", (MC, 128, HPC, 128), f32r, kind="ExternalInput")
    bqs_d = nc.dram_tensor("bqs", (D,), f32, kind="ExternalInput")
    bk_d = nc.dram_tensor("bk", (D,), f32, kind="ExternalInput")
    bvr_d = nc.dram_tensor("bvr", (1, D), f32r, kind="ExternalInput")
    bo_d = nc.dram_tensor("bo", (D,), f32, kind="ExternalInput")
    ones1_d = nc.dram_tensor("ones1", (1, 128), f32r, kind="ExternalInput")
    onescol_d = nc.dram_tensor("onescol", (128, 1), f32r, kind="ExternalInput")
    out_d = nc.dram_tensor("out", (D, S), f32, kind="ExternalOutput")

    with tile.TileContext(nc) as tc, ExitStack() as top:
        rpool = top.enter_context(tc.tile_pool(name="consts", bufs=1))
        dpool = top.enter_context(tc.tile_pool(name="dram", bufs=1, space="DRAM"))

        bq_sb = rpool.tile([128, MC], f32)
        bk_sb = rpool.tile([128, MC], f32)
        bo_sb = rpool.tile([128, MC], f32)
        bv_sb = rpool.tile([1, D], f32r)
        ones1 = rpool.tile([1, 128], f32r)
        onescol = rpool.tile([128, 1], f32r)
        nc.sync.dma_start(bq_sb[:], bqs_d.ap().rearrange("(t p) -> p t", p=128))
        nc.sync.dma_start(bk_sb[:], bk_d.ap().rearrange("(t p) -> p t", p=128))
        nc.sync.dma_start(bo_sb[:], bo_d.ap().rearrange("(t p) -> p t", p=128))
        nc.sync.dma_start(bv_sb[:], bvr_d.ap())
        nc.sync.dma_start(ones1[:], ones1_d.ap())
        nc.sync.dma_start(onescol[:], onescol_d.ap())

        qhat_dram = dpool.tile([MC, 128, RPC], f32r)        # [t][dk][r]
        khat_dram = dpool.tile([HPC, 128, MC, 128], f32r)   # [h][dk][tk][u]
        vhat_dram = dpool.tile([RPC, D], f32r)              # natural R_v

        def load_stage(pool, src_d):
            st = pool.tile([128, MC, RPC], f32r, tag="st")
            for mc in range(MC):
                nc.sync.dma_start(st[:, mc, :], src_d.ap()[mc])
            return st

        def load_w16(pool, src_ap, tag):
            """Load a [128, MC, 128] weight tile in 4 mc-chunks."""
            st = pool.tile([128, MC, 128], f32r, tag=tag)
            for g in range(4):
                nc.sync.dma_start(st[:, 4 * g:4 * g + 4, :],
                                  src_ap[:, 4 * g:4 * g + 4, :])
            return st

        with ExitStack() as stages_es:
            st_pool = stages_es.enter_context(tc.tile_pool(name="stages", bufs=2))
            w_pool = stages_es.enter_context(tc.tile_pool(name="weights", bufs=3))

            # ============= phase Q (first; weights load from t=0) ======
            with ExitStack() as ph:
                qps_pool = ph.enter_context(
                    tc.tile_pool(name="qps", bufs=4, space="PSUM"))
                qout_pool = ph.enter_context(tc.tile_pool(name="qout", bufs=4))
                with nc.named_scope("proj_q"):
                    qt_st = load_stage(st_pool, qts_d)
                    vt_st = load_stage(st_pool, vts_d)   # prefetch for V
                    for ct in range(MC):
                        wq_st = load_w16(w_pool, wqt_d.ap()[ct], "w")
                        for rb in range(2):
                            ps = qps_pool.tile([128, 512], f32, tag="qps")
                            for mc in range(MC):
                                nc.tensor.matmul(
                                    ps[:], wq_st[:, mc, :],
                                    qt_st[:, mc, 512 * rb:512 * rb + 512],
                                    start=(mc == 0), stop=(mc == MC - 1))
                            qo = qout_pool.tile([128, 512], f32r, tag="qo")
                            nc.scalar.activation(qo[:], ps[:], AF.Identity,
                                                 bias=bq_sb[:, ct:ct + 1],
                                                 scale=SCALE)
                            nc.gpsimd.dma_start(
                                qhat_dram[ct, :, 512 * rb:512 * rb + 512],
                                qo[:])

            # ============= phase V (+ prefetch K stage) ================
            with ExitStack() as ph:
                vps_pool = ph.enter_context(
                    tc.tile_pool(name="vps", bufs=4, space="PSUM"))
                vout_pool = ph.enter_context(tc.tile_pool(name="vout", bufs=4))
                with nc.named_scope("proj_v"):
                    kt_st = load_stage(st_pool, kts_d)   # prefetch (rotates)
                    for cb in range(8):           # c blocks of 256
                        wv_st = w_pool.tile([128, MC, 256], f32r, tag="w")
                        for g in range(4):
                            nc.sync.dma_start(
                                wv_st[:, 4 * g:4 * g + 4, :],
                                wvt_d.ap()[cb][:, 4 * g:4 * g + 4, :])
                        for rt in range(8):       # r tiles of 128
                            ps = vps_pool.tile([128, 256], f32, tag="vps")
                            for mc in range(MC):
                                nc.tensor.matmul(
                                    ps[:],
                                    vt_st[:, mc, 128 * rt:128 * rt + 128],
                                    wv_st[:, mc, :], start=(mc == 0),
                                    stop=False)
                            nc.tensor.matmul(
                                ps[:], ones1[:],
                                bv_sb[:, 256 * cb:256 * cb + 256],
                                start=False, stop=True)
                            vo = vout_pool.tile([128, 256], f32r, tag="vo")
                            nc.vector.tensor_copy(vo[:], ps[:])
                            nc.gpsimd.dma_start(
                                vhat_dram[128 * rt:128 * rt + 128,
                                          256 * cb:256 * cb + 256], vo[:])

            # ============= phase K -> khat_dram [h][dk][tk][u] =========
            with ExitStack() as ph:
                kps_pool = ph.enter_context(
                    tc.tile_pool(name="kps", bufs=4, space="PSUM"))
                kout_pool = ph.enter_context(tc.tile_pool(name="kout", bufs=4))
                with nc.named_scope("proj_k"):
                    for ct in range(MC):
                        wk_st = load_w16(w_pool, wkt_d.ap()[ct], "w")
                        for rb in range(2):
                            ps = kps_pool.tile([128, 512], f32, tag="kps")
                            for mc in range(MC):
                                nc.tensor.matmul(
                                    ps[:], wk_st[:, mc, :],
                                    kt_st[:, mc, 512 * rb:512 * rb + 512],
                                    start=(mc == 0), stop=(mc == MC - 1))
                            ko = kout_pool.tile([128, 4, 128], f32r, tag="ko")
                            nc.scalar.activation(ko[:], ps[:], AF.Identity,
                                                 bias=bk_sb[:, ct:ct + 1],
                                                 scale=1.0)
                            dst = khat_dram[4 * rb:4 * rb + 4, :, ct, :] \
                                .rearrange("h p u -> p h u")
                            nc.gpsimd.dma_start(dst, ko[:])

        # ============= attention + output projection ===============
        with ExitStack() as ph:
            q_pool = ph.enter_context(tc.tile_pool(name="qrhs", bufs=4))
            kh_pool = ph.enter_context(tc.tile_pool(name="kh", bufs=3))
            vh_pool = ph.enter_context(tc.tile_pool(name="vh", bufs=3))
            exp_pool = ph.enter_context(tc.tile_pool(name="expp", bufs=6))
            tree_pool = ph.enter_context(tc.tile_pool(name="tree", bufs=2))
            scps_pool = ph.enter_context(
                tc.tile_pool(name="scps", bufs=2, space="PSUM"))
            xps_pool = ph.enter_context(
                tc.tile_pool(name="xps", bufs=2, space="PSUM"))
            sps_pool = ph.enter_context(
                tc.tile_pool(name="sps", bufs=1, space="PSUM"))
            ops_pool = ph.enter_context(
                tc.tile_pool(name="ops", bufs=1, space="PSUM"))
            nrm_pool = ph.enter_context(tc.tile_pool(name="nrm", bufs=2))
            x_pool = ph.enter_context(tc.tile_pool(name="xsb", bufs=3))
            wo_pool = ph.enter_context(tc.tile_pool(name="wo", bufs=3))
            oout_pool = ph.enter_context(tc.tile_pool(name="oout", bufs=3))
            with nc.named_scope("attn"):
                NP_ = MC // 2     # key-tile pairs per head

                def attn_block(j, h, x_j, k_h, v_h):
                    q_rhs = q_pool.tile([128, 4, 128], f32r, tag="qr")
                    nc.sync.dma_start(
                        q_rhs[:],
                        qhat_dram[4 * j:4 * j + 4, :, 128 * h:128 * h + 128]
                        .rearrange("t p u -> p t u"))
                    x_ps = xps_pool.tile([128, 512], f32, tag="xps")
                    s_ps = sps_pool.tile([1, 512], f32, tag="sps")
                    acc = tree_pool.tile([128, 512], f32, tag="acc")
                    tsum = tree_pool.tile([128, 512], f32r, tag="tf")
                    exs = [None] * NP_
                    t1s = [None] * NP_

                    def pv_and_sum(tp):
                        ex = exs[tp]
                        for i in range(2):
                            nc.tensor.matmul(
                                x_ps[:], v_h[:, 2 * tp + i, :],
                                ex[:, i, :], start=(tp == 0 and i == 0),
                                stop=(tp == NP_ - 1 and i == 1))
                        t1 = tree_pool.tile([128, 512], f32, tag="t1")
                        nc.vector.tensor_add(
                            t1[:], ex[:, 0, :].bitcast(f32),
                            ex[:, 1, :].bitcast(f32))
                        t1s[tp] = t1
                        if tp == 1:
                            nc.vector.tensor_add(acc[:], t1s[0][:], t1[:])
                        elif 1 < tp < NP_ - 1:
                            nc.vector.tensor_add(acc[:], acc[:], t1[:])
                        elif tp == NP_ - 1:
                            nc.vector.tensor_add(tsum[:], acc[:], t1[:])

                    for tp in range(NP_):
                        sc = scps_pool.tile([128, 2, 512], f32, tag="sc")
                        for i in range(2):
                            tk = 2 * tp + i
                            nc.tensor.matmul(
                                sc[:, i, :], k_h[:, tk, :],
                                q_rhs[:, :, :], start=True, stop=True)
                        ex = exp_pool.tile([128, 2, 512], f32r, tag="ex")
                        nc.scalar.activation(ex[:], sc[:], AF.Exp, scale=1.0)
                        exs[tp] = ex
                        if tp >= 1:
                            pv_and_sum(tp - 1)
                    pv_and_sum(NP_ - 1)

                    nc.tensor.matmul(s_ps[:], onescol[:], tsum[:],
                                     start=True, stop=True)
                    rec = nrm_pool.tile([1, 512], f32, tag="rec")
                    nc.vector.reciprocal_approx_fast(rec[:], s_ps[:])
                    bcast = nrm_pool.tile([128, 512], f32, tag="bc")
                    nc.gpsimd.partition_broadcast(bcast[:], rec[:])
                    nc.vector.tensor_mul(x_j[:, h, :], x_ps[:], bcast[:])

                def emit_outproj(j, x_j):
                    for ot in range(MC):
                        wo_st = wo_pool.tile([128, HPC, 128], f32r, tag="wo")
                        for g in range(2):
                            nc.sync.dma_start(
                                wo_st[:, 4 * g:4 * g + 4, :],
                                wot_d.ap()[ot][:, 4 * g:4 * g + 4, :])
                        op = ops_pool.tile([128, 512], f32, tag="op")
                        for h in range(HPC):
                            nc.tensor.matmul(op[:], wo_st[:, h, :],
                                             x_j[:, h, :], start=(h == 0),
                                             stop=(h == HPC - 1))
                        oo = oout_pool.tile([128, 512], f32, tag="oo")
                        nc.scalar.activation(oo[:], op[:], AF.Identity,
                                             bias=bo_sb[:, ot:ot + 1],
                                             scale=1.0)
                        nc.gpsimd.dma_start(
                            out_d.ap()[128 * ot:128 * ot + 128,
                                       512 * j:512 * j + 512], oo[:])

                for jp in range(2):           # pairs of query pi-blocks
                    j0, j1 = 2 * jp, 2 * jp + 1
                    x_j0 = x_pool.tile([128, HPC, 512], f32r, tag="xj")
                    x_j1 = x_pool.tile([128, HPC, 512], f32r, tag="xj")
                    for h in range(HPC):
                        k_h = kh_pool.tile([128, MC, 128], f32r, tag="kh")
                        for g in range(4):
                            nc.sync.dma_start(
                                k_h[:, 4 * g:4 * g + 4, :],
                                khat_dram[h][:, 4 * g:4 * g + 4, :])
                        v_h = vh_pool.tile([128, MC, 128], f32r, tag="vh")
                        for g in range(4):
                            nc.sync.dma_start(
                                v_h[:, 4 * g:4 * g + 4, :],
                                vhat_dram[128 * h:128 * h + 128,
                                          512 * g:512 * g + 512])
                        attn_block(j0, h, x_j0, k_h, v_h)
                        attn_block(j1, h, x_j1, k_h, v_h)
                    emit_outproj(j0, x_j0)
                    emit_outproj(j1, x_j1)

    nc.compile()
    return nc


def _prep_shared(Wq, Wk, Wv, Wo, bq, bk, bv, bo):
    wqt = _round_f32r(np.ascontiguousarray(np.asarray(Wq, np.float32).T))
    wkt = _round_f32r(np.ascontiguousarray(np.asarray(Wk, np.float32).T))
    wvt = _round_f32r(np.ascontiguousarray(np.asarray(Wv, np.float32).T))
    wqt_t = np.ascontiguousarray(
        wqt.reshape(MC, 128, MC, 128).transpose(2, 1, 0, 3))
    wkt_t = np.ascontiguousarray(
        wkt.reshape(MC, 128, MC, 128).transpose(2, 1, 0, 3))
    wvt_t = np.ascontiguousarray(
        wvt.reshape(MC, 128, 8, 256).transpose(2, 1, 0, 3))
    woT = np.ascontiguousarray(np.asarray(Wo, np.float32).T)
    bqs = (np.asarray(bq, np.float32) * SCALE).copy()
    bk_np = np.asarray(bk, np.float32).copy()
    bvr = _round_f32r(np.asarray(bv, np.float32).reshape(1, D))
    bo_np = np.asarray(bo, np.float32).copy()
    return wqt_t, wkt_t, wvt_t, woT, bqs, bk_np, bvr, bo_np


def kernel(Q, K, V, Wq, bq, Wk, bk, Wv, bv, Wo, bo, num_heads):
    global last_results
    assert int(num_heads) == H

    from concourse.bass_utils import run_bass_kernel_spmd

    if "nc" not in _cache:
        _cache["nc"] = _build()
    nc = _cache["nc"]

    Q = np.asarray(Q, np.float32)
    K = np.asarray(K, np.float32)
    V = np.asarray(V, np.float32)
    wqt_t, wkt_t, wvt_t, woT, bqs, bk_np, bvr, bo_np = _prep_shared(
        Wq, Wk, Wv, Wo, bq, bk, bv, bo)
    ones1 = np.ones((1, 128), np.float32)
    onescol = np.ones((128, 1), np.float32)

    in_maps = []
    for c in range(NC_):
        b, half = divmod(c, 2)
        r0 = RPC * half
        wot_t = np.ascontiguousarray(
            _round_f32r(woT[r0:r0 + RPC, :])
            .reshape(HPC, 128, MC, 128).transpose(2, 1, 0, 3))
        in_maps.append({
            "qts": _round_f32r(Q[b].T[:, r0:r0 + RPC]).reshape(MC, 128, RPC),
            "kts": _round_f32r(K[b].T[:, r0:r0 + RPC]).reshape(MC, 128, RPC),
            "vts": _round_f32r(V[b].T[:, r0:r0 + RPC]).reshape(MC, 128, RPC),
            "wqt": wqt_t, "wkt": wkt_t, "wvt": wvt_t, "wot": wot_t,
            "bqs": bqs, "bk": bk_np, "bvr": bvr, "bo": bo_np,
            "ones1": ones1, "onescol": onescol,
        })

    res = run_bass_kernel_spmd(nc, in_maps, core_ids=list(range(NC_)))
    last_results = res

    out = np.empty((B, S, D), np.float32)
    for b in range(B):
        oT = res.results[2 * b]["out"] + res.results[2 * b + 1]["out"]
        # oT[o, pi], pi = 128*t + u ; s = 16*u + t
        out[b] = oT.reshape(D, 16, 128).transpose(2, 1, 0).reshape(S, D)
    return out
